# revision 27
# baseline (speedup 1.0000x reference)
"""Self-attention of Q against itself (K, V ignored), B=4, S=2048, H=16, D=64.

Sharding: 64 independent (batch, head) attention instances, 8 per core.
Core k handles batch b = k//2, heads hg*8..hg*8+8 where hg = k%2, so each
core's input is the contiguous block Q[b, :, hg*512:(hg+1)*512] ([2048, 512]).

Host-side marshalling (part of kernel()): the per-core shard is pre-cast to
bf16 and shipped in the two layouts the device needs, so no on-device casts,
transposes, or q1 assembly are required:
  QT [4*128, 2048]: per pair pr, rows pr*128+(x*64+d) = q^T (head A rows
     0-63, head B rows 64-127), columns s.
  Q1 [4*128, 16*130]: per pair, [s%128, s//128, x*65+d] with d=64 column
     set to 1.0 (the softmax-denominator trick).

Heads are processed in pairs (A, B): head A occupies PE row-groups 0-1 and
head B row-groups 2-3 for the K=64 score matmuls (concurrent), and the two
heads' score strips share one [128, 1024] psum tile so a single ACT exp op
covers the pair.

exp(scores) is symmetric, so only the upper-triangle chunks are computed and
exp'd (40 of 64 [128 x 1024] strips per pair); the lower-triangle tiles are
obtained by transposing exp'd tiles with the DMA XBAR transposer (one
dma_start(transpose=True) flips all 8 [128,128] blocks of a chunk), which
costs no compute-engine time.  Chunks are processed in DESCENDING window
order so every mirror's source chunk is exp'd before the mirror is needed.

Per chunk c (window of 4 column tiles), strips J = 0..4c+3:
  sc chunk [128, 1024] = two concurrent bf16 matmuls  qt_J^T @ qt_window
  exp -> es[J] bf16 (ACT, the bottleneck engine)
  c > J//4: XBAR-mirror es[J] -> m[J][c] for later chunks
After the strips, 8 AV "column bursts": out[s in tile 4c+i, d] accumulates
16 matmuls  es-or-mirror-tile^T @ (q_J | 1)  in psum [128, 65]; col 64 is
the softmax denominator (column sums == row sums by symmetry).  Burst
matmuls are emitted in half-burst granularity interleaved with later strips;
the Tile scheduler orders per-engine work by readiness + emission priority,
so what matters is the dependency graph (pool sizes) plus emission order as
a priority hint.
"""

import os
import sys

import numpy as np

if os.path.isdir("/opt/trn_rl_repo"):
    sys.path.insert(0, "/opt/trn_rl_repo")

import concourse.bass as bass  # noqa: E402
import concourse.mybir as mybir  # noqa: E402
import concourse.tile as tile  # noqa: E402
from concourse import bacc  # noqa: E402
from concourse.bass_utils import run_bass_kernel_spmd  # noqa: E402

B, S, DMODEL = 4, 2048, 1024
NHEAD, D = 16, 64
P = 128
NT = S // P  # 16 t-tiles of 128
HPC = 8  # heads per core
NPAIR = HPC // 2
N_CORES = 8
CW = 512  # s-chunk width
NC_CHUNK = S // CW  # 4

F32 = mybir.dt.float32
BF16 = mybir.dt.bfloat16
EXP = mybir.ActivationFunctionType.Exp
MULT = mybir.AluOpType.mult


def _emit(tc: tile.TileContext, Out: bass.AP, Qt: bass.AP, Q1: bass.AP,
          loop_n: int = 1, probe: str = ""):
    nc = tc.nc
    probes = set(probe.split(",")) if probe else set()
    Qt_r = Qt.rearrange("(r p) s -> p r s", p=P)   # [128, 4, 2048]
    Q1_r = Q1.rearrange("(r p) m -> p r m", p=P)   # [128, 4, 2080]
    # Out rows = 512*c + 128*j + p, cols = h*64 + d
    Out_r = Out.rearrange("(c j p) m -> c p j m", j=4, p=P)  # [4, 128, 4, 512]

    with (
        tc.tile_pool(name="qtp", bufs=3) as qtp,
        tc.tile_pool(name="q1p", bufs=3) as q1p,
        tc.tile_pool(name="esp", bufs=20) as esp,
        tc.tile_pool(name="mirp", bufs=26) as mirp,
        tc.tile_pool(name="osbp", bufs=3) as osbp,
        tc.tile_pool(name="recp", bufs=4) as recp,
        tc.tile_pool(name="ztp", bufs=1) as ztp,
        tc.tile_pool(name="ps_sc", bufs=2, space="PSUM") as ps_sc,
        tc.tile_pool(name="ps_av", bufs=4, space="PSUM") as ps_av,
    ):
        def make_prep(pr, st, lead=False):
            """DMA pieces that load qt/q1 for pair pr into dict st.

            lead=True loads the chunk-3 window quarter (s-tiles 12-15)
            first so the rotated strip order (J=12 first) can start after
            a quarter of the qt load.
            """
            qgrp = S // 4

            def p_qt(g):
                if st.get("qt") is None:
                    st["qt"] = qtp.tile([P, S], BF16, tag="qt",
                                        name=f"qt{pr}")
                sl = slice(g * qgrp, (g + 1) * qgrp)
                nc.sync.dma_start(st["qt"][:, sl], Qt_r[:, pr, sl])

            def p_q1(h):
                if h == 0:
                    st["q1"] = q1p.tile([P, NT, 2 * (D + 1)], BF16, tag="q1",
                                        name=f"q1{pr}")
                half = NT // 2
                sl = slice(h * half, (h + 1) * half)
                w = 2 * (D + 1) * half
                nc.sync.dma_start(
                    st["q1"][:, sl, :],
                    Q1_r[:, pr, h * w : (h + 1) * w],
                )

            pieces = []
            order = (3, 2, 0, 1) if lead else (0, 1, 2, 3)
            for g in order:
                pieces.append(lambda g=g: p_qt(g))
            pieces.append(lambda: p_q1(0))
            pieces.append(lambda: p_q1(1))
            return pieces

        def body():
            states = [dict() for _ in range(NPAIR)]
            # pending: list of (weight, closure). weight 1 = PE-heavy (burst
            # half, ~8 matmuls), weight 0 = light (prep piece / out tail:
            # DMA/DVE only, no PE work).
            pending = []
            # Warm the exp table-set while the first DMAs stream: a 1-elem
            # activation forces LoadActFuncSet off the critical path.
            warm = recp.tile([P, 4], F32, tag="rec", name="warm")
            nc.vector.memset(warm[:, 0:1], 0.0)
            nc.scalar.activation(warm[:, 1:2], warm[:, 0:1], EXP)
            # zero row for the av-bank group openers (K=1 zero matmul)
            zt = ztp.tile([1, 4 * (D + 1)], BF16, tag="zt", name="zt")
            nc.vector.memset(zt[:], 0.0)
            for pc in make_prep(0, states[0], lead=True):
                pc()  # pair 0 prep upfront
            if NPAIR > 1:
                pending = [(0, pc) for pc in make_prep(1, states[1])]

            es = {}   # (pr, c, J) -> es chunk
            mir = {}  # (pr, J, c_src) -> mirror tile [128, 8, 128]

            def drain(nheavy):
                # Pop until nheavy heavy items done (or queue empty); light
                # items in between are free (not PE work).
                done = 0
                nlight = 0
                while pending and done < nheavy and nlight < 4:
                    w, pc = pending.pop(0)
                    pc()
                    if w:
                        done += 1
                    else:
                        nlight += 1

            for pr in range(NPAIR):
                st = states[pr]
                qt2, q1 = st["qt"], st["q1"]  # qt2 [128, 2048]

                def emit_sc(c, J):
                    sc = ps_sc.tile([P, 1024], F32, tag="sc")
                    if "noscores" not in probes:
                        for x in range(2):
                            nc.tensor.matmul(
                                sc[:, x * CW : (x + 1) * CW],
                                qt2[x * D : (x + 1) * D, J * P : (J + 1) * P],
                                qt2[x * D : (x + 1) * D, c * CW : (c + 1) * CW],
                                start=True,
                                stop=True,
                            )
                    else:
                        nc.vector.memset(sc[:, 0:1], 1.0)
                    return sc

                def av_mm(apr, aq1, c, x, i, av, Jp, stop=False):
                    # One AV accumulate: out[s in tile 4c+i, d-or-denom] +=
                    # A[t in Jp, s]^T @ (q_Jp | 1).  Direct es for Jp <=
                    # 4c+3, XBAR mirror for Jp >= 4c+4.  The bank's group
                    # was opened by the zero-opener, so order-free.
                    # apr/aq1 are bound at piece creation: mirror pieces may
                    # pop while a later pair's loop variables are current.
                    if Jp <= 4 * c + 3:
                        lhsT = es[apr, c, Jp][
                            :, x * CW + i * P : x * CW + (i + 1) * P
                        ]
                    else:
                        lhsT = mir[apr, 4 * c + i, Jp // 4][
                            :, x * 4 + Jp % 4, :
                        ]
                    nc.tensor.matmul(
                        av[:, i, 0 : D + 1],
                        lhsT,
                        aq1[:, Jp, x * (D + 1) : (x + 1) * (D + 1)],
                        start=False,
                        stop=stop,
                        skip_group_check=True,
                    )

                def make_mir_piece(apr, aq1, c, x, i, av, c_src):
                    def piece():
                        for Jp in range(4 * c_src, 4 * c_src + 4):
                            av_mm(apr, aq1, c, x, i, av, Jp)

                    return piece

                def out_half(pr, c, x, av, box):
                    # av [128, 4, 65-used]: rows = s in tile 4c+i, col 64 =
                    # softmax denom. Normalize into the pair's osb tile.
                    def tail():
                        if x == 0:
                            osb = osbp.tile([P, 4, 2 * D], F32, tag="osb",
                                            name=f"osb{pr}_{c}")
                            box["osb"] = osb
                        osb = box["osb"]
                        rec = recp.tile([P, 4], F32, tag="rec")
                        nc.vector.reciprocal(rec[:], av[:, :, D])
                        for jj in range(4):
                            nc.vector.tensor_scalar(
                                osb[:, jj, x * D : (x + 1) * D],
                                av[:, jj, 0:D],
                                rec[:, jj : jj + 1], None, op0=MULT,
                            )

                    return tail

                def out_flush(pr, c, box):
                    # Combined 2-head DMA (512B runs).
                    def flush():
                        nc.sync.dma_start(
                            Out_r[c, :, :, 2 * pr * D : (2 * pr + 2) * D],
                            box["osb"],
                        )

                    return flush

                for c in range(NC_CHUNK - 1, -1, -1):
                    strips = list(range(4 * c + 4))
                    if c == NC_CHUNK - 1:
                        # start at J=12 so the first sc only needs qt qtr 3
                        strips = strips[12:] + strips[8:12] + strips[:8]
                    do_av = "noav" not in probes
                    if do_av:
                        av_a = ps_av.tile([P, 4, P], F32, tag="av",
                                          name=f"avA{pr}{c}")
                        av_b = ps_av.tile([P, 4, P], F32, tag="av",
                                          name=f"avB{pr}{c}")
                        for av in (av_a, av_b):
                            # zero-opener: one K=1 matmul spanning all four
                            # quarters opens the bank's accumulation group;
                            # every av_mm overlaps its bytes, so the
                            # scheduler orders the opener first.
                            nc.tensor.matmul(
                                av[:, :, 0 : D + 1],
                                zt[0:1, 0:P],
                                zt[0:1, 0 : 4 * (D + 1)],
                                start=True,
                                stop=False,
                                skip_group_check=True,
                            )
                        # mirror-part MMs (rows Jp >= 4c+4): their mirrors
                        # exist since chunks > c are already done, so these
                        # pieces are ready at chunk start and pad PE slack.
                        for c_src in range(c + 1, NC_CHUNK):
                            for x, av in ((0, av_a), (1, av_b)):
                                for i in range(4):
                                    pending.append((1, make_mir_piece(
                                        pr, q1, c, x, i, av, c_src)))
                    sc = emit_sc(c, strips[0])
                    for k, J in enumerate(strips):
                        est = esp.tile([P, 1024], BF16, tag="es",
                                       name=f"es{pr}_{c}_{J}")
                        es[pr, c, J] = est
                        if "noexp" not in probes:
                            nc.scalar.activation(est[:], sc[:], EXP,
                                                 scale=0.125)
                        else:
                            nc.vector.memset(est[:, 0:1], 1.0)
                        if k + 1 < len(strips):
                            sc = emit_sc(c, strips[k + 1])
                        if do_av:
                            # direct AV matmuls for this strip, right behind
                            # the next sc in priority order
                            last = k == len(strips) - 1
                            for x, av in ((0, av_a), (1, av_b)):
                                for i in range(4):
                                    av_mm(pr, q1, c, x, i, av, J,
                                          stop=last and i == 3)
                        nheavy = sum(w for w, _ in pending)
                        drain(2 if nheavy > 6 else 1)
                        if c > J // 4:
                            # mirror whole chunk: 8 block-transposes in one
                            # XBAR DMA -> tiles [4c+i, J] for later chunks
                            m = mirp.tile([P, 8, P], BF16, tag="mir",
                                          name=f"m{pr}_{J}_{c}")
                            mir[pr, J, c] = m
                            nc.sync.dma_start(m[:], est[:], transpose=True)
                    if do_av:
                        box = {}
                        # tails pop after the mirror pieces (FIFO) so all av
                        # writers are emitted before the readers
                        pending.append((0, out_half(pr, c, 0, av_a, box)))
                        pending.append((0, out_half(pr, c, 1, av_b, box)))
                        pending.append((0, out_flush(pr, c, box)))
                if pr + 2 < NPAIR:
                    pending.extend(
                        (0, pc) for pc in make_prep(pr + 2, states[pr + 2]))
            for _, pc in pending:
                pc()

        if loop_n > 1:
            with tc.For_i(0, loop_n, 1):
                body()
        else:
            body()


_CACHED = {}


def _build(loop_n: int = 1, probe: str = ""):
    key = (loop_n, probe)
    if key in _CACHED:
        return _CACHED[key]
    nc = bacc.Bacc("TRN2", target_bir_lowering=False, debug=False)
    Qt = nc.dram_tensor("Qt", [NPAIR * P, S], BF16, kind="ExternalInput")
    Q1 = nc.dram_tensor("Q1", [NPAIR * P, NT * 2 * (D + 1)], BF16,
                        kind="ExternalInput")
    Out = nc.dram_tensor("Out", [S, HPC * D], F32, kind="ExternalOutput")
    with tile.TileContext(nc) as tc:
        _emit(tc, Out.ap(), Qt.ap(), Q1.ap(), loop_n=loop_n, probe=probe)
    nc.compile()
    _CACHED[key] = nc
    return nc


def _bf16(a):
    import ml_dtypes

    return np.asarray(a, dtype=np.float32).astype(ml_dtypes.bfloat16)


def prep_core(Q: np.ndarray, core: int) -> dict:
    """Host-side marshalling of core `core`'s shard into QT/Q1 layouts."""
    b, hg = core // 2, core % 2
    qc = np.asarray(Q[b, :, hg * 512 : (hg + 1) * 512], dtype=np.float32)
    # QT: [pr*128 + x*64 + d, s]
    qt = np.ascontiguousarray(qc.T)  # [512, 2048]
    # Q1: [pr*128 + p, n*130 + x*65 + dd], dd==64 -> 1.0
    q1 = np.ones((NPAIR, P, NT, 2, D + 1), dtype=np.float32)
    qr = qc.reshape(NT, P, NPAIR, 2, D)  # [n, p, pr, x, d]
    q1[:, :, :, :, :D] = qr.transpose(2, 1, 0, 3, 4)
    return {
        "Qt": _bf16(qt),
        "Q1": _bf16(q1.reshape(NPAIR * P, NT * 2 * (D + 1))),
    }


def kernel(Q: np.ndarray, K: np.ndarray, V: np.ndarray,
           _probe: str = "") -> np.ndarray:
    nc = _build(1, _probe)
    in_maps = [prep_core(Q, core) for core in range(N_CORES)]
    res = run_bass_kernel_spmd(nc, in_maps, list(range(N_CORES))).results
    out = np.empty((B, S, DMODEL), np.float32)
    for core in range(N_CORES):
        b, hg = core // 2, core % 2
        out[b, :, hg * 512 : (hg + 1) * 512] = res[core]["Out"]
    return out


# revision 32
# speedup vs baseline: 1.6386x; 1.6386x over previous
"""Self-attention of Q against itself (K, V ignored), B=4, S=2048, H=16, D=64.

Sharding: 64 independent (batch, head) attention instances, 8 per core.
Core k handles batch b = k//2, heads hg*8..hg*8+8 where hg = k%2, so each
core's input is the contiguous block Q[b, :, hg*512:(hg+1)*512] ([2048, 512]).

Host-side marshalling (part of kernel()): the per-core shard is pre-cast to
bf16 and shipped in the two layouts the device needs, so no on-device casts,
transposes, or q1 assembly are required:
  QT [4*128, 2048]: per pair pr, rows pr*128+(x*64+d) = q^T (head A rows
     0-63, head B rows 64-127), columns s.
  Q1 [4*128, 16*130]: per pair, [s%128, s//128, x*65+d] with d=64 column
     set to 1.0 (the softmax-denominator trick).

Heads are processed in pairs (A, B): head A occupies PE row-groups 0-1 and
head B row-groups 2-3 for the K=64 score matmuls (concurrent), and the two
heads' score strips share one [128, 1024] psum tile so a single ACT exp op
covers the pair.

exp(scores) is symmetric, so only the upper-triangle chunks are computed and
exp'd (40 of 64 [128 x 1024] strips per pair); the lower-triangle tiles are
obtained by transposing exp'd tiles with the DMA XBAR transposer (one
dma_start(transpose=True) flips all 8 [128,128] blocks of a chunk), which
costs no compute-engine time.  Chunks are processed in DESCENDING window
order so every mirror's source chunk is exp'd before the mirror is needed.

Per chunk c (window of 4 column tiles), strips J = 0..4c+3:
  sc chunk [128, 1024] = two concurrent bf16 matmuls  qt_J^T @ qt_window
  exp -> es[J] bf16 (ACT, the bottleneck engine)
  c > J//4: XBAR-mirror es[J] -> m[J][c] for later chunks
After the strips, 8 AV "column bursts": out[s in tile 4c+i, d] accumulates
16 matmuls  es-or-mirror-tile^T @ (q_J | 1)  in psum [128, 65]; col 64 is
the softmax denominator (column sums == row sums by symmetry).  Burst
matmuls are emitted in half-burst granularity interleaved with later strips;
the Tile scheduler orders per-engine work by readiness + emission priority,
so what matters is the dependency graph (pool sizes) plus emission order as
a priority hint.
"""

import os
import sys

import numpy as np

if os.path.isdir("/opt/trn_rl_repo"):
    sys.path.insert(0, "/opt/trn_rl_repo")

import concourse.bass as bass  # noqa: E402
import concourse.mybir as mybir  # noqa: E402
import concourse.tile as tile  # noqa: E402
from concourse import bacc  # noqa: E402
from concourse.bass_utils import run_bass_kernel_spmd  # noqa: E402

B, S, DMODEL = 4, 2048, 1024
NHEAD, D = 16, 64
P = 128
NT = S // P  # 16 t-tiles of 128
HPC = 8  # heads per core
NPAIR = HPC // 2
N_CORES = 8
CW = 512  # s-chunk width
NC_CHUNK = S // CW  # 4

F32 = mybir.dt.float32
BF16 = mybir.dt.bfloat16
EXP = mybir.ActivationFunctionType.Exp
MULT = mybir.AluOpType.mult

# AV scheduling mode: defer (classic whole-burst groups drained behind later
# strips; faster on HW) vs inline per-strip accumulation (slower on HW).
AV_DEFER = os.environ.get("K_AV_DEFER", "1") == "1"


def _emit(tc: tile.TileContext, Out: bass.AP, Qt: bass.AP, Q1: bass.AP,
          loop_n: int = 1, probe: str = ""):
    nc = tc.nc
    probes = set(probe.split(",")) if probe else set()
    Qt_r = Qt.rearrange("(r p) s -> p r s", p=P)   # [128, 4, 2048]
    Q1_r = Q1.rearrange("(r p) m -> p r m", p=P)   # [128, 4, 2080]
    # Out rows = 512*c + 128*j + p, cols = h*64 + d
    Out_r = Out.rearrange("(c j p) m -> c p j m", j=4, p=P)  # [4, 128, 4, 512]

    with (
        tc.tile_pool(name="qtp", bufs=3) as qtp,
        tc.tile_pool(name="q1p", bufs=3) as q1p,
        tc.tile_pool(name="esp", bufs=20) as esp,
        tc.tile_pool(name="mirp", bufs=26) as mirp,
        tc.tile_pool(name="osbp", bufs=3) as osbp,
        tc.tile_pool(name="recp", bufs=4) as recp,
        tc.tile_pool(name="ztp", bufs=1) as ztp,
        tc.tile_pool(name="ps_sc", bufs=2, space="PSUM") as ps_sc,
        tc.tile_pool(name="ps_av", bufs=4, space="PSUM") as ps_av,
    ):
        def make_prep(pr, st, lead=False):
            """DMA pieces that load qt/q1 for pair pr into dict st.

            lead=True loads the chunk-3 window quarter (s-tiles 12-15)
            first so the rotated strip order (J=12 first) can start after
            a quarter of the qt load.
            """
            qgrp = S // 4

            def p_qt(g):
                if st.get("qt") is None:
                    st["qt"] = qtp.tile([P, S], BF16, tag="qt",
                                        name=f"qt{pr}")
                sl = slice(g * qgrp, (g + 1) * qgrp)
                nc.sync.dma_start(st["qt"][:, sl], Qt_r[:, pr, sl])

            def p_q1(h):
                if h == 0:
                    st["q1"] = q1p.tile([P, NT, 2 * (D + 1)], BF16, tag="q1",
                                        name=f"q1{pr}")
                half = NT // 2
                sl = slice(h * half, (h + 1) * half)
                w = 2 * (D + 1) * half
                nc.sync.dma_start(
                    st["q1"][:, sl, :],
                    Q1_r[:, pr, h * w : (h + 1) * w],
                )

            pieces = []
            order = (3, 2, 0, 1) if lead else (0, 1, 2, 3)
            for g in order:
                pieces.append(lambda g=g: p_qt(g))
            pieces.append(lambda: p_q1(0))
            pieces.append(lambda: p_q1(1))
            return pieces

        def body():
            states = [dict() for _ in range(NPAIR)]
            # pending: list of (weight, closure). weight 1 = PE-heavy (burst
            # half, ~8 matmuls), weight 0 = light (prep piece / out tail:
            # DMA/DVE only, no PE work).
            pending = []
            # Warm the exp table-set while the first DMAs stream: a 1-elem
            # activation forces LoadActFuncSet off the critical path.
            warm = recp.tile([P, 4], F32, tag="rec", name="warm")
            nc.vector.memset(warm[:, 0:1], 0.0)
            nc.scalar.activation(warm[:, 1:2], warm[:, 0:1], EXP)
            # zero row for the av-bank group openers (K=1 zero matmul)
            zt = ztp.tile([1, 4 * (D + 1)], BF16, tag="zt", name="zt")
            nc.vector.memset(zt[:], 0.0)
            for pc in make_prep(0, states[0], lead=True):
                pc()  # pair 0 prep upfront
            if NPAIR > 1:
                pending = [(0, pc) for pc in make_prep(1, states[1])]

            es = {}   # (pr, c, J) -> es chunk
            mir = {}  # (pr, J, c_src) -> mirror tile [128, 8, 128]

            def drain(nheavy):
                # Pop until nheavy heavy items done (or queue empty); light
                # items in between are free (not PE work).
                done = 0
                nlight = 0
                while pending and done < nheavy and nlight < 4:
                    w, pc = pending.pop(0)
                    pc()
                    if w:
                        done += 1
                    else:
                        nlight += 1

            for pr in range(NPAIR):
                st = states[pr]
                qt2, q1 = st["qt"], st["q1"]  # qt2 [128, 2048]

                def emit_sc(c, J):
                    sc = ps_sc.tile([P, 1024], F32, tag="sc")
                    if "noscores" not in probes:
                        for x in range(2):
                            nc.tensor.matmul(
                                sc[:, x * CW : (x + 1) * CW],
                                qt2[x * D : (x + 1) * D, J * P : (J + 1) * P],
                                qt2[x * D : (x + 1) * D, c * CW : (c + 1) * CW],
                                start=True,
                                stop=True,
                            )
                    else:
                        nc.vector.memset(sc[:, 0:1], 1.0)
                    return sc

                def av_mm(apr, aq1, c, x, i, av, Jp, stop=False):
                    # One AV accumulate: out[s in tile 4c+i, d-or-denom] +=
                    # A[t in Jp, s]^T @ (q_Jp | 1).  Direct es for Jp <=
                    # 4c+3, XBAR mirror for Jp >= 4c+4.  The bank's group
                    # was opened by the zero-opener, so order-free.
                    # apr/aq1 are bound at piece creation: mirror pieces may
                    # pop while a later pair's loop variables are current.
                    if Jp <= 4 * c + 3:
                        lhsT = es[apr, c, Jp][
                            :, x * CW + i * P : x * CW + (i + 1) * P
                        ]
                    else:
                        lhsT = mir[apr, 4 * c + i, Jp // 4][
                            :, x * 4 + Jp % 4, :
                        ]
                    nc.tensor.matmul(
                        av[:, i, 0 : D + 1],
                        lhsT,
                        aq1[:, Jp, x * (D + 1) : (x + 1) * (D + 1)],
                        start=False,
                        stop=stop,
                        skip_group_check=True,
                    )

                def make_mir_piece(apr, aq1, c, x, i, av, c_src):
                    def piece():
                        for Jp in range(4 * c_src, 4 * c_src + 4):
                            av_mm(apr, aq1, c, x, i, av, Jp)

                    return piece

                def make_burst_half(apr, c, x, i, av, aq1, jlo, jhi):
                    # defer mode: classic 16-MM accumulation group per
                    # (x, i), split in two queue items; groups on a bank
                    # stay sequential (FIFO pops), start/stop per group.
                    def burst():
                        for Jp in range(jlo, jhi):
                            if Jp <= 4 * c + 3:
                                lhsT = es[apr, c, Jp][
                                    :, x * CW + i * P : x * CW + (i + 1) * P
                                ]
                            else:
                                lhsT = mir[apr, 4 * c + i, Jp // 4][
                                    :, x * 4 + Jp % 4, :
                                ]
                            nc.tensor.matmul(
                                av[:, i, 0 : D + 1],
                                lhsT,
                                aq1[:, Jp, x * (D + 1) : (x + 1) * (D + 1)],
                                start=(Jp == 0),
                                stop=(Jp == NT - 1),
                            )

                    return burst

                def out_half(pr, c, x, av, box):
                    # av [128, 4, 65-used]: rows = s in tile 4c+i, col 64 =
                    # softmax denom. Normalize into the pair's osb tile.
                    def tail():
                        if x == 0:
                            osb = osbp.tile([P, 4, 2 * D], F32, tag="osb",
                                            name=f"osb{pr}_{c}")
                            box["osb"] = osb
                        osb = box["osb"]
                        rec = recp.tile([P, 4], F32, tag="rec")
                        nc.vector.reciprocal(rec[:], av[:, :, D])
                        for jj in range(4):
                            nc.vector.tensor_scalar(
                                osb[:, jj, x * D : (x + 1) * D],
                                av[:, jj, 0:D],
                                rec[:, jj : jj + 1], None, op0=MULT,
                            )

                    return tail

                def out_flush(pr, c, box):
                    # Combined 2-head DMA (512B runs).
                    def flush():
                        nc.sync.dma_start(
                            Out_r[c, :, :, 2 * pr * D : (2 * pr + 2) * D],
                            box["osb"],
                        )

                    return flush

                for c in range(NC_CHUNK - 1, -1, -1):
                    strips = list(range(4 * c + 4))
                    if c == NC_CHUNK - 1:
                        # start at J=12 so the first sc only needs qt qtr 3
                        strips = strips[12:] + strips[8:12] + strips[:8]
                    do_av = "noav" not in probes
                    if do_av:
                        av_a = ps_av.tile([P, 4, P], F32, tag="av",
                                          name=f"avA{pr}{c}")
                        av_b = ps_av.tile([P, 4, P], F32, tag="av",
                                          name=f"avB{pr}{c}")
                        if not AV_DEFER:
                            for av in (av_a, av_b):
                                # zero-opener: one K=1 matmul spanning all
                                # four quarters opens the bank's group;
                                # every av_mm overlaps its bytes, so the
                                # scheduler orders the opener first.
                                nc.tensor.matmul(
                                    av[:, :, 0 : D + 1],
                                    zt[0:1, 0:P],
                                    zt[0:1, 0 : 4 * (D + 1)],
                                    start=True,
                                    stop=False,
                                    skip_group_check=True,
                                )
                            # mirror-part MMs (rows Jp >= 4c+4): mirrors
                            # exist since chunks > c are done, so these
                            # pieces are ready at chunk start.
                            for c_src in range(c + 1, NC_CHUNK):
                                for x, av in ((0, av_a), (1, av_b)):
                                    for i in range(4):
                                        pending.append((1, make_mir_piece(
                                            pr, q1, c, x, i, av, c_src)))
                    sc = emit_sc(c, strips[0])
                    for k, J in enumerate(strips):
                        est = esp.tile([P, 1024], BF16, tag="es",
                                       name=f"es{pr}_{c}_{J}")
                        es[pr, c, J] = est
                        if "noexp" not in probes:
                            nc.scalar.activation(est[:], sc[:], EXP,
                                                 scale=0.125)
                        else:
                            nc.vector.memset(est[:, 0:1], 1.0)
                        if k + 1 < len(strips):
                            sc = emit_sc(c, strips[k + 1])
                        if do_av and not AV_DEFER:
                            # direct AV matmuls for this strip, right behind
                            # the next sc in priority order
                            last = k == len(strips) - 1
                            for x, av in ((0, av_a), (1, av_b)):
                                for i in range(4):
                                    av_mm(pr, q1, c, x, i, av, J,
                                          stop=last and i == 3)
                        nheavy = sum(w for w, _ in pending)
                        drain(3 if nheavy > 14 else 2 if nheavy > 6 else 1)
                        if c > J // 4:
                            # mirror whole chunk: 8 block-transposes in one
                            # XBAR DMA -> tiles [4c+i, J] for later chunks
                            m = mirp.tile([P, 8, P], BF16, tag="mir",
                                          name=f"m{pr}_{J}_{c}")
                            mir[pr, J, c] = m
                            nc.sync.dma_start(m[:], est[:], transpose=True)
                    if do_av:
                        if AV_DEFER:
                            for x, av in ((0, av_a), (1, av_b)):
                                for i in range(4):
                                    for jlo in (0, NT // 2):
                                        pending.append((1, make_burst_half(
                                            pr, c, x, i, av, q1,
                                            jlo, jlo + NT // 2)))
                        box = {}
                        # tails pop after the av writers (FIFO) so all
                        # writers are emitted before the readers
                        pending.append((0, out_half(pr, c, 0, av_a, box)))
                        pending.append((0, out_half(pr, c, 1, av_b, box)))
                        pending.append((0, out_flush(pr, c, box)))
                if pr + 2 < NPAIR:
                    pending.extend(
                        (0, pc) for pc in make_prep(pr + 2, states[pr + 2]))
            for _, pc in pending:
                pc()

        if loop_n > 1:
            with tc.For_i(0, loop_n, 1):
                body()
        else:
            body()


_CACHED = {}


def _build(loop_n: int = 1, probe: str = ""):
    key = (loop_n, probe)
    if key in _CACHED:
        return _CACHED[key]
    nc = bacc.Bacc("TRN2", target_bir_lowering=False, debug=False)
    Qt = nc.dram_tensor("Qt", [NPAIR * P, S], BF16, kind="ExternalInput")
    Q1 = nc.dram_tensor("Q1", [NPAIR * P, NT * 2 * (D + 1)], BF16,
                        kind="ExternalInput")
    Out = nc.dram_tensor("Out", [S, HPC * D], F32, kind="ExternalOutput")
    with tile.TileContext(nc) as tc:
        _emit(tc, Out.ap(), Qt.ap(), Q1.ap(), loop_n=loop_n, probe=probe)
    nc.compile()
    _CACHED[key] = nc
    return nc


def _bf16(a):
    import ml_dtypes

    return np.asarray(a, dtype=np.float32).astype(ml_dtypes.bfloat16)


def prep_core(Q: np.ndarray, core: int) -> dict:
    """Host-side marshalling of core `core`'s shard into QT/Q1 layouts."""
    b, hg = core // 2, core % 2
    qc = np.asarray(Q[b, :, hg * 512 : (hg + 1) * 512], dtype=np.float32)
    # QT: [pr*128 + x*64 + d, s]
    qt = np.ascontiguousarray(qc.T)  # [512, 2048]
    # Q1: [pr*128 + p, n*130 + x*65 + dd], dd==64 -> 1.0
    q1 = np.ones((NPAIR, P, NT, 2, D + 1), dtype=np.float32)
    qr = qc.reshape(NT, P, NPAIR, 2, D)  # [n, p, pr, x, d]
    q1[:, :, :, :, :D] = qr.transpose(2, 1, 0, 3, 4)
    return {
        "Qt": _bf16(qt),
        "Q1": _bf16(q1.reshape(NPAIR * P, NT * 2 * (D + 1))),
    }


def kernel(Q: np.ndarray, K: np.ndarray, V: np.ndarray,
           _probe: str = "") -> np.ndarray:
    nc = _build(1, _probe)
    in_maps = [prep_core(Q, core) for core in range(N_CORES)]
    res = run_bass_kernel_spmd(nc, in_maps, list(range(N_CORES))).results
    out = np.empty((B, S, DMODEL), np.float32)
    for core in range(N_CORES):
        b, hg = core // 2, core % 2
        out[b, :, hg * 512 : (hg + 1) * 512] = res[core]["Out"]
    return out


# revision 33
# speedup vs baseline: 1.6457x; 1.0043x over previous
"""Self-attention of Q against itself (K, V ignored), B=4, S=2048, H=16, D=64.

Sharding: 64 independent (batch, head) attention instances, 8 per core.
Core k handles batch b = k//2, heads hg*8..hg*8+8 where hg = k%2, so each
core's input is the contiguous block Q[b, :, hg*512:(hg+1)*512] ([2048, 512]).

Host-side marshalling (part of kernel()): the per-core shard is pre-cast to
bf16 and shipped in the two layouts the device needs, so no on-device casts,
transposes, or q1 assembly are required:
  QT [4*128, 2048]: per pair pr, rows pr*128+(x*64+d) = q^T (head A rows
     0-63, head B rows 64-127), columns s.
  Q1 [4*128, 16*130]: per pair, [s%128, s//128, x*65+d] with d=64 column
     set to 1.0 (the softmax-denominator trick).

Heads are processed in pairs (A, B): head A occupies PE row-groups 0-1 and
head B row-groups 2-3 for the K=64 score matmuls (concurrent), and the two
heads' score strips share one [128, 1024] psum tile so a single ACT exp op
covers the pair.

exp(scores) is symmetric, so only the upper-triangle chunks are computed and
exp'd (40 of 64 [128 x 1024] strips per pair); the lower-triangle tiles are
obtained by transposing exp'd tiles with the DMA XBAR transposer (one
dma_start(transpose=True) flips all 8 [128,128] blocks of a chunk), which
costs no compute-engine time.  Chunks are processed in DESCENDING window
order so every mirror's source chunk is exp'd before the mirror is needed.

Per chunk c (window of 4 column tiles), strips J = 0..4c+3:
  sc chunk [128, 1024] = two concurrent bf16 matmuls  qt_J^T @ qt_window
  exp -> es[J] bf16 (ACT, the bottleneck engine)
  c > J//4: XBAR-mirror es[J] -> m[J][c] for later chunks
After the strips, 8 AV "column bursts": out[s in tile 4c+i, d] accumulates
16 matmuls  es-or-mirror-tile^T @ (q_J | 1)  in psum [128, 65]; col 64 is
the softmax denominator (column sums == row sums by symmetry).  Burst
matmuls are emitted in half-burst granularity interleaved with later strips;
the Tile scheduler orders per-engine work by readiness + emission priority,
so what matters is the dependency graph (pool sizes) plus emission order as
a priority hint.
"""

import os
import sys

import numpy as np

if os.path.isdir("/opt/trn_rl_repo"):
    sys.path.insert(0, "/opt/trn_rl_repo")

import concourse.bass as bass  # noqa: E402
import concourse.mybir as mybir  # noqa: E402
import concourse.tile as tile  # noqa: E402
from concourse import bacc  # noqa: E402
from concourse.bass_utils import run_bass_kernel_spmd  # noqa: E402

B, S, DMODEL = 4, 2048, 1024
NHEAD, D = 16, 64
P = 128
NT = S // P  # 16 t-tiles of 128
HPC = 8  # heads per core
NPAIR = HPC // 2
N_CORES = 8
CW = 512  # s-chunk width
NC_CHUNK = S // CW  # 4

F32 = mybir.dt.float32
BF16 = mybir.dt.bfloat16
EXP = mybir.ActivationFunctionType.Exp
MULT = mybir.AluOpType.mult

# AV scheduling mode: defer (classic whole-burst groups drained behind later
# strips; faster on HW) vs inline per-strip accumulation (slower on HW).
AV_DEFER = os.environ.get("K_AV_DEFER", "1") == "1"


def _emit(tc: tile.TileContext, Out: bass.AP, Qt: bass.AP, Q1: bass.AP,
          loop_n: int = 1, probe: str = ""):
    nc = tc.nc
    probes = set(probe.split(",")) if probe else set()
    Qt_r = Qt.rearrange("(r p) s -> p r s", p=P)   # [128, 4, 2048]
    Q1_r = Q1.rearrange("(r p) m -> p r m", p=P)   # [128, 4, 2080]
    # Out rows = 512*c + 128*j + p, cols = h*64 + d
    Out_r = Out.rearrange("(c j p) m -> c p j m", j=4, p=P)  # [4, 128, 4, 512]

    with (
        tc.tile_pool(name="qtp", bufs=3) as qtp,
        tc.tile_pool(name="q1p", bufs=3) as q1p,
        tc.tile_pool(name="esp", bufs=36) as esp,
        tc.tile_pool(name="mirp", bufs=26) as mirp,
        tc.tile_pool(name="osbp", bufs=3) as osbp,
        tc.tile_pool(name="recp", bufs=4) as recp,
        tc.tile_pool(name="ztp", bufs=1) as ztp,
        tc.tile_pool(name="ps_sc", bufs=2, space="PSUM") as ps_sc,
        tc.tile_pool(name="ps_av", bufs=4, space="PSUM") as ps_av,
    ):
        def make_prep(pr, st, lead=False):
            """DMA pieces that load qt/q1 for pair pr into dict st.

            lead=True loads the chunk-3 window quarter (s-tiles 12-15)
            first so the rotated strip order (J=12 first) can start after
            a quarter of the qt load.
            """
            qgrp = S // 4

            def p_qt(g):
                if st.get("qt") is None:
                    st["qt"] = qtp.tile([P, S], BF16, tag="qt",
                                        name=f"qt{pr}")
                sl = slice(g * qgrp, (g + 1) * qgrp)
                nc.sync.dma_start(st["qt"][:, sl], Qt_r[:, pr, sl])

            def p_q1(h):
                if h == 0:
                    st["q1"] = q1p.tile([P, NT, 2 * (D + 1)], BF16, tag="q1",
                                        name=f"q1{pr}")
                half = NT // 2
                sl = slice(h * half, (h + 1) * half)
                w = 2 * (D + 1) * half
                nc.sync.dma_start(
                    st["q1"][:, sl, :],
                    Q1_r[:, pr, h * w : (h + 1) * w],
                )

            pieces = []
            order = (3, 2, 0, 1) if lead else (0, 1, 2, 3)
            for g in order:
                pieces.append(lambda g=g: p_qt(g))
            pieces.append(lambda: p_q1(0))
            pieces.append(lambda: p_q1(1))
            return pieces

        def body():
            states = [dict() for _ in range(NPAIR)]
            # pending: list of (weight, closure). weight 1 = PE-heavy (burst
            # half, ~8 matmuls), weight 0 = light (prep piece / out tail:
            # DMA/DVE only, no PE work).
            pending = []
            # Warm the exp table-set while the first DMAs stream: a 1-elem
            # activation forces LoadActFuncSet off the critical path.
            warm = recp.tile([P, 4], F32, tag="rec", name="warm")
            nc.vector.memset(warm[:, 0:1], 0.0)
            nc.scalar.activation(warm[:, 1:2], warm[:, 0:1], EXP)
            # zero row for the av-bank group openers (K=1 zero matmul)
            zt = ztp.tile([1, 4 * (D + 1)], BF16, tag="zt", name="zt")
            nc.vector.memset(zt[:], 0.0)
            for pc in make_prep(0, states[0], lead=True):
                pc()  # pair 0 prep upfront
            if NPAIR > 1:
                pending = [(0, pc) for pc in make_prep(1, states[1])]

            es = {}   # (pr, c, J) -> es chunk
            mir = {}  # (pr, J, c_src) -> mirror tile [128, 8, 128]

            def drain(nheavy):
                # Pop until nheavy heavy items done (or queue empty); light
                # items in between are free (not PE work).
                done = 0
                nlight = 0
                while pending and done < nheavy and nlight < 4:
                    w, pc = pending.pop(0)
                    pc()
                    if w:
                        done += 1
                    else:
                        nlight += 1

            for pr in range(NPAIR):
                st = states[pr]
                qt2, q1 = st["qt"], st["q1"]  # qt2 [128, 2048]

                def emit_sc(c, J):
                    sc = ps_sc.tile([P, 1024], F32, tag="sc")
                    if "noscores" not in probes:
                        for x in range(2):
                            nc.tensor.matmul(
                                sc[:, x * CW : (x + 1) * CW],
                                qt2[x * D : (x + 1) * D, J * P : (J + 1) * P],
                                qt2[x * D : (x + 1) * D, c * CW : (c + 1) * CW],
                                start=True,
                                stop=True,
                            )
                    else:
                        nc.vector.memset(sc[:, 0:1], 1.0)
                    return sc

                def av_mm(apr, aq1, c, x, i, av, Jp, stop=False):
                    # One AV accumulate: out[s in tile 4c+i, d-or-denom] +=
                    # A[t in Jp, s]^T @ (q_Jp | 1).  Direct es for Jp <=
                    # 4c+3, XBAR mirror for Jp >= 4c+4.  The bank's group
                    # was opened by the zero-opener, so order-free.
                    # apr/aq1 are bound at piece creation: mirror pieces may
                    # pop while a later pair's loop variables are current.
                    if Jp <= 4 * c + 3:
                        lhsT = es[apr, c, Jp][
                            :, x * CW + i * P : x * CW + (i + 1) * P
                        ]
                    else:
                        lhsT = mir[apr, 4 * c + i, Jp // 4][
                            :, x * 4 + Jp % 4, :
                        ]
                    nc.tensor.matmul(
                        av[:, i, 0 : D + 1],
                        lhsT,
                        aq1[:, Jp, x * (D + 1) : (x + 1) * (D + 1)],
                        start=False,
                        stop=stop,
                        skip_group_check=True,
                    )

                def make_mir_piece(apr, aq1, c, x, i, av, c_src):
                    def piece():
                        for Jp in range(4 * c_src, 4 * c_src + 4):
                            av_mm(apr, aq1, c, x, i, av, Jp)

                    return piece

                def make_burst_half(apr, c, x, i, av, aq1, jlo, jhi):
                    # defer mode: classic 16-MM accumulation group per
                    # (x, i), split in two queue items; groups on a bank
                    # stay sequential (FIFO pops), start/stop per group.
                    def burst():
                        for Jp in range(jlo, jhi):
                            if Jp <= 4 * c + 3:
                                lhsT = es[apr, c, Jp][
                                    :, x * CW + i * P : x * CW + (i + 1) * P
                                ]
                            else:
                                lhsT = mir[apr, 4 * c + i, Jp // 4][
                                    :, x * 4 + Jp % 4, :
                                ]
                            nc.tensor.matmul(
                                av[:, i, 0 : D + 1],
                                lhsT,
                                aq1[:, Jp, x * (D + 1) : (x + 1) * (D + 1)],
                                start=(Jp == 0),
                                stop=(Jp == NT - 1),
                            )

                    return burst

                def out_half(pr, c, x, av, box):
                    # av [128, 4, 65-used]: rows = s in tile 4c+i, col 64 =
                    # softmax denom. Normalize into the pair's osb tile.
                    def tail():
                        if x == 0:
                            osb = osbp.tile([P, 4, 2 * D], F32, tag="osb",
                                            name=f"osb{pr}_{c}")
                            box["osb"] = osb
                        osb = box["osb"]
                        rec = recp.tile([P, 4], F32, tag="rec")
                        nc.vector.reciprocal(rec[:], av[:, :, D])
                        for jj in range(4):
                            nc.vector.tensor_scalar(
                                osb[:, jj, x * D : (x + 1) * D],
                                av[:, jj, 0:D],
                                rec[:, jj : jj + 1], None, op0=MULT,
                            )

                    return tail

                def out_flush(pr, c, box):
                    # Combined 2-head DMA (512B runs).
                    def flush():
                        nc.sync.dma_start(
                            Out_r[c, :, :, 2 * pr * D : (2 * pr + 2) * D],
                            box["osb"],
                        )

                    return flush

                for c in range(NC_CHUNK - 1, -1, -1):
                    strips = list(range(4 * c + 4))
                    if c == NC_CHUNK - 1:
                        # start at J=12 so the first sc only needs qt qtr 3
                        strips = strips[12:] + strips[8:12] + strips[:8]
                    do_av = "noav" not in probes
                    if do_av:
                        av_a = ps_av.tile([P, 4, P], F32, tag="av",
                                          name=f"avA{pr}{c}")
                        av_b = ps_av.tile([P, 4, P], F32, tag="av",
                                          name=f"avB{pr}{c}")
                        if not AV_DEFER:
                            for av in (av_a, av_b):
                                # zero-opener: one K=1 matmul spanning all
                                # four quarters opens the bank's group;
                                # every av_mm overlaps its bytes, so the
                                # scheduler orders the opener first.
                                nc.tensor.matmul(
                                    av[:, :, 0 : D + 1],
                                    zt[0:1, 0:P],
                                    zt[0:1, 0 : 4 * (D + 1)],
                                    start=True,
                                    stop=False,
                                    skip_group_check=True,
                                )
                            # mirror-part MMs (rows Jp >= 4c+4): mirrors
                            # exist since chunks > c are done, so these
                            # pieces are ready at chunk start.
                            for c_src in range(c + 1, NC_CHUNK):
                                for x, av in ((0, av_a), (1, av_b)):
                                    for i in range(4):
                                        pending.append((1, make_mir_piece(
                                            pr, q1, c, x, i, av, c_src)))
                    sc = emit_sc(c, strips[0])
                    for k, J in enumerate(strips):
                        est = esp.tile([P, 1024], BF16, tag="es",
                                       name=f"es{pr}_{c}_{J}")
                        es[pr, c, J] = est
                        if "noexp" not in probes:
                            nc.scalar.activation(est[:], sc[:], EXP,
                                                 scale=0.125)
                        else:
                            nc.vector.memset(est[:, 0:1], 1.0)
                        if k + 1 < len(strips):
                            sc = emit_sc(c, strips[k + 1])
                        if do_av and not AV_DEFER:
                            # direct AV matmuls for this strip, right behind
                            # the next sc in priority order
                            last = k == len(strips) - 1
                            for x, av in ((0, av_a), (1, av_b)):
                                for i in range(4):
                                    av_mm(pr, q1, c, x, i, av, J,
                                          stop=last and i == 3)
                        nheavy = sum(w for w, _ in pending)
                        drain(3 if nheavy > 14 else 2 if nheavy > 6 else 1)
                        if c > J // 4:
                            # mirror whole chunk: 8 block-transposes in one
                            # XBAR DMA -> tiles [4c+i, J] for later chunks
                            m = mirp.tile([P, 8, P], BF16, tag="mir",
                                          name=f"m{pr}_{J}_{c}")
                            mir[pr, J, c] = m
                            nc.sync.dma_start(m[:], est[:], transpose=True)
                    if do_av:
                        if AV_DEFER:
                            for x, av in ((0, av_a), (1, av_b)):
                                for i in range(4):
                                    for jlo in (0, NT // 2):
                                        pending.append((1, make_burst_half(
                                            pr, c, x, i, av, q1,
                                            jlo, jlo + NT // 2)))
                        box = {}
                        # tails pop after the av writers (FIFO) so all
                        # writers are emitted before the readers
                        pending.append((0, out_half(pr, c, 0, av_a, box)))
                        pending.append((0, out_half(pr, c, 1, av_b, box)))
                        pending.append((0, out_flush(pr, c, box)))
                if pr + 2 < NPAIR:
                    pending.extend(
                        (0, pc) for pc in make_prep(pr + 2, states[pr + 2]))
            for _, pc in pending:
                pc()

        if loop_n > 1:
            with tc.For_i(0, loop_n, 1):
                body()
        else:
            body()


_CACHED = {}


def _build(loop_n: int = 1, probe: str = ""):
    key = (loop_n, probe)
    if key in _CACHED:
        return _CACHED[key]
    nc = bacc.Bacc("TRN2", target_bir_lowering=False, debug=False)
    Qt = nc.dram_tensor("Qt", [NPAIR * P, S], BF16, kind="ExternalInput")
    Q1 = nc.dram_tensor("Q1", [NPAIR * P, NT * 2 * (D + 1)], BF16,
                        kind="ExternalInput")
    Out = nc.dram_tensor("Out", [S, HPC * D], F32, kind="ExternalOutput")
    with tile.TileContext(nc) as tc:
        _emit(tc, Out.ap(), Qt.ap(), Q1.ap(), loop_n=loop_n, probe=probe)
    nc.compile()
    _CACHED[key] = nc
    return nc


def _bf16(a):
    import ml_dtypes

    return np.asarray(a, dtype=np.float32).astype(ml_dtypes.bfloat16)


def prep_core(Q: np.ndarray, core: int) -> dict:
    """Host-side marshalling of core `core`'s shard into QT/Q1 layouts."""
    b, hg = core // 2, core % 2
    qc = np.asarray(Q[b, :, hg * 512 : (hg + 1) * 512], dtype=np.float32)
    # QT: [pr*128 + x*64 + d, s]
    qt = np.ascontiguousarray(qc.T)  # [512, 2048]
    # Q1: [pr*128 + p, n*130 + x*65 + dd], dd==64 -> 1.0
    q1 = np.ones((NPAIR, P, NT, 2, D + 1), dtype=np.float32)
    qr = qc.reshape(NT, P, NPAIR, 2, D)  # [n, p, pr, x, d]
    q1[:, :, :, :, :D] = qr.transpose(2, 1, 0, 3, 4)
    return {
        "Qt": _bf16(qt),
        "Q1": _bf16(q1.reshape(NPAIR * P, NT * 2 * (D + 1))),
    }


def kernel(Q: np.ndarray, K: np.ndarray, V: np.ndarray,
           _probe: str = "") -> np.ndarray:
    nc = _build(1, _probe)
    in_maps = [prep_core(Q, core) for core in range(N_CORES)]
    res = run_bass_kernel_spmd(nc, in_maps, list(range(N_CORES))).results
    out = np.empty((B, S, DMODEL), np.float32)
    for core in range(N_CORES):
        b, hg = core // 2, core % 2
        out[b, :, hg * 512 : (hg + 1) * 512] = res[core]["Out"]
    return out


# revision 46
# speedup vs baseline: 1.8213x; 1.1067x over previous
"""Self-attention of Q against itself (K, V ignored), B=4, S=2048, H=16, D=64.

Sharding: 64 independent (batch, head) attention instances, 8 per core.
Core k handles batch b = k//2, heads hg*8..hg*8+8 where hg = k%2, so each
core's input is the contiguous block Q[b, :, hg*512:(hg+1)*512] ([2048, 512]).

Host-side marshalling (part of kernel()): the per-core shard is pre-cast to
bf16 and shipped in the two layouts the device needs, so no on-device casts,
transposes, or q1 assembly are required:
  QT [4*128, 2048]: per pair pr, rows pr*128+(x*64+d) = q^T (head A rows
     0-63, head B rows 64-127), columns s.
  Q1 [4*128, 16*130]: per pair, [s%128, s//128, x*65+d] with d=64 column
     set to 1.0 (the softmax-denominator trick).

Heads are processed in pairs (A, B): head A occupies PE row-groups 0-1 and
head B row-groups 2-3 for the K=64 score matmuls (concurrent), and the two
heads' score strips share one [128, 1024] psum tile so a single ACT exp op
covers the pair.

exp(scores) is symmetric, so only the upper-triangle chunks are computed and
exp'd (40 of 64 [128 x 1024] strips per pair); the lower-triangle tiles are
obtained by transposing exp'd tiles with the DMA XBAR transposer (one
dma_start(transpose=True) flips all 8 [128,128] blocks of a chunk), which
costs no compute-engine time.  Chunks are processed in DESCENDING window
order so every mirror's source chunk is exp'd before the mirror is needed.

Per chunk c (window of 4 column tiles), strips J = 0..4c+3:
  sc chunk [128, 1024] = two concurrent bf16 matmuls  qt_J^T @ qt_window
  exp -> es[J] bf16 (ACT, the bottleneck engine)
  c > J//4: XBAR-mirror es[J] -> m[J][c] for later chunks
After the strips, 8 AV "column bursts": out[s in tile 4c+i, d] accumulates
16 matmuls  es-or-mirror-tile^T @ (q_J | 1)  in psum [128, 65]; col 64 is
the softmax denominator (column sums == row sums by symmetry).  Burst
matmuls are emitted in half-burst granularity interleaved with later strips;
the Tile scheduler orders per-engine work by readiness + emission priority,
so what matters is the dependency graph (pool sizes) plus emission order as
a priority hint.
"""

import os
import sys

import numpy as np

if os.path.isdir("/opt/trn_rl_repo"):
    sys.path.insert(0, "/opt/trn_rl_repo")

import concourse.bass as bass  # noqa: E402
import concourse.mybir as mybir  # noqa: E402
import concourse.tile as tile  # noqa: E402
from concourse import bacc  # noqa: E402
from concourse.bass_utils import run_bass_kernel_spmd  # noqa: E402

B, S, DMODEL = 4, 2048, 1024
NHEAD, D = 16, 64
P = 128
NT = S // P  # 16 t-tiles of 128
HPC = 8  # heads per core
NPAIR = HPC // 2
N_CORES = 8
CW = 512  # s-chunk width
NC_CHUNK = S // CW  # 4

F32 = mybir.dt.float32
BF16 = mybir.dt.bfloat16
EXP = mybir.ActivationFunctionType.Exp
MULT = mybir.AluOpType.mult

# AV scheduling mode: defer (classic whole-burst groups drained behind later
# strips; faster on HW) vs inline per-strip accumulation (slower on HW).
AV_DEFER = os.environ.get("K_AV_DEFER", "1") == "1"


def _emit(tc: tile.TileContext, Out: bass.AP, Qt: bass.AP, Q1: bass.AP,
          loop_n: int = 1, probe: str = ""):
    nc = tc.nc
    probes = set(probe.split(",")) if probe else set()
    Qt_r = Qt.rearrange("(r p) s -> p r s", p=P)   # [128, 4, 2048]
    Q1_r = Q1.rearrange("(r p) m -> p r m", p=P)   # [128, 4, 2080]
    # Out rows = 512*c + 128*j + p, cols = h*64 + d
    Out_r = Out.rearrange("(c j p) m -> c p j m", j=4, p=P)  # [4, 128, 4, 512]

    with (
        tc.tile_pool(name="qtp", bufs=3) as qtp,
        tc.tile_pool(name="q1p", bufs=3) as q1p,
        tc.tile_pool(name="esp", bufs=36) as esp,
        tc.tile_pool(name="mirp", bufs=26) as mirp,
        tc.tile_pool(name="osbp", bufs=3) as osbp,
        tc.tile_pool(name="recp", bufs=4) as recp,
        tc.tile_pool(name="ztp", bufs=1) as ztp,
        tc.tile_pool(name="ps_sc", bufs=2, space="PSUM") as ps_sc,
        tc.tile_pool(name="ps_av", bufs=4, space="PSUM") as ps_av,
    ):
        def make_prep(pr, st, lead=False):
            """DMA pieces that load qt/q1 for pair pr into dict st.

            lead=True loads the chunk-3 window quarter (s-tiles 12-15)
            first so the rotated strip order (J=12 first) can start after
            a quarter of the qt load.
            """
            qgrp = S // 4

            def p_qt(g):
                if st.get("qt") is None:
                    st["qt"] = qtp.tile([P, S], BF16, tag="qt",
                                        name=f"qt{pr}")
                sl = slice(g * qgrp, (g + 1) * qgrp)
                nc.sync.dma_start(st["qt"][:, sl], Qt_r[:, pr, sl])

            def p_q1(h):
                if h == 0:
                    st["q1"] = q1p.tile([P, NT, 2 * (D + 1)], BF16, tag="q1",
                                        name=f"q1{pr}")
                half = NT // 2
                sl = slice(h * half, (h + 1) * half)
                w = 2 * (D + 1) * half
                nc.sync.dma_start(
                    st["q1"][:, sl, :],
                    Q1_r[:, pr, h * w : (h + 1) * w],
                )

            pieces = []
            order = (3, 2, 0, 1) if lead else (0, 1, 2, 3)
            for g in order:
                pieces.append(lambda g=g: p_qt(g))
            pieces.append(lambda: p_q1(0))
            pieces.append(lambda: p_q1(1))
            return pieces

        def body():
            states = [dict() for _ in range(NPAIR)]
            # pending: list of (weight, closure). weight 1 = PE-heavy (burst
            # half, ~8 matmuls), weight 0 = light (prep piece / out tail:
            # DMA/DVE only, no PE work).
            pending = []
            # Warm the exp table-set while the first DMAs stream: a 1-elem
            # activation forces LoadActFuncSet off the critical path.
            warm = recp.tile([P, 4], F32, tag="rec", name="warm")
            nc.vector.memset(warm[:, 0:1], 0.0)
            nc.scalar.activation(warm[:, 1:2], warm[:, 0:1], EXP)
            # zero row for the av-bank group openers (K=1 zero matmul)
            zt = ztp.tile([1, 4 * (D + 1)], BF16, tag="zt", name="zt")
            nc.vector.memset(zt[:], 0.0)
            for pc in make_prep(0, states[0], lead=True):
                pc()  # pair 0 prep upfront
            if NPAIR > 1:
                pending = [(0, pc) for pc in make_prep(1, states[1])]

            es = {}   # (pr, c, J) -> es chunk
            mir = {}  # (pr, J, c_src) -> mirror tile [128, 8, 128]

            def drain(nheavy):
                # Pop until nheavy heavy items done (or queue empty); light
                # items in between are free (not PE work).
                done = 0
                nlight = 0
                while pending and done < nheavy and nlight < 4:
                    w, pc = pending.pop(0)
                    pc()
                    if w:
                        done += 1
                    else:
                        nlight += 1

            for pr in range(NPAIR):
                st = states[pr]
                qt2, q1 = st["qt"], st["q1"]  # qt2 [128, 2048]

                def emit_sc(c, J):
                    sc = ps_sc.tile([P, 1024], F32, tag="sc")
                    if "noscores" not in probes:
                        for x in range(2):
                            nc.tensor.matmul(
                                sc[:, x * CW : (x + 1) * CW],
                                qt2[x * D : (x + 1) * D, J * P : (J + 1) * P],
                                qt2[x * D : (x + 1) * D, c * CW : (c + 1) * CW],
                                start=True,
                                stop=True,
                            )
                    else:
                        nc.vector.memset(sc[:, 0:1], 1.0)
                    return sc

                def av_mm(apr, aq1, c, x, i, av, Jp, stop=False):
                    # One AV accumulate: out[s in tile 4c+i, d-or-denom] +=
                    # A[t in Jp, s]^T @ (q_Jp | 1).  Direct es for Jp <=
                    # 4c+3, XBAR mirror for Jp >= 4c+4.  The bank's group
                    # was opened by the zero-opener, so order-free.
                    # apr/aq1 are bound at piece creation: mirror pieces may
                    # pop while a later pair's loop variables are current.
                    if Jp <= 4 * c + 3:
                        lhsT = es[apr, c, Jp][
                            :, x * CW + i * P : x * CW + (i + 1) * P
                        ]
                    else:
                        lhsT = mir[apr, 4 * c + i, Jp // 4][
                            :, x * 4 + Jp % 4, :
                        ]
                    nc.tensor.matmul(
                        av[:, i, 0 : D + 1],
                        lhsT,
                        aq1[:, Jp, x * (D + 1) : (x + 1) * (D + 1)],
                        start=False,
                        stop=stop,
                        skip_group_check=True,
                    )

                def make_mir_piece(apr, aq1, c, x, i, av, c_src):
                    def piece():
                        for Jp in range(4 * c_src, 4 * c_src + 4):
                            av_mm(apr, aq1, c, x, i, av, Jp)

                    return piece

                def make_burst_half(apr, c, x, i, av, aq1, jlo, jhi):
                    # defer mode: classic 16-MM accumulation group per
                    # (x, i), split in two queue items; groups on a bank
                    # stay sequential (FIFO pops), start/stop per group.
                    def burst():
                        for Jp in range(jlo, jhi):
                            if Jp <= 4 * c + 3:
                                lhsT = es[apr, c, Jp][
                                    :, x * CW + i * P : x * CW + (i + 1) * P
                                ]
                            else:
                                lhsT = mir[apr, 4 * c + i, Jp // 4][
                                    :, x * 4 + Jp % 4, :
                                ]
                            nc.tensor.matmul(
                                av[:, i, 0 : D + 1],
                                lhsT,
                                aq1[:, Jp, x * (D + 1) : (x + 1) * (D + 1)],
                                start=(Jp == 0),
                                stop=(Jp == NT - 1),
                            )

                    return burst

                def out_half(pr, c, x, av, box):
                    # av [128, 4, 65-used]: rows = s in tile 4c+i, col 64 =
                    # softmax denom. Normalize into the pair's osb tile.
                    def tail():
                        if x == 0:
                            osb = osbp.tile([P, 4, 2 * D], F32, tag="osb",
                                            name=f"osb{pr}_{c}")
                            box["osb"] = osb
                        osb = box["osb"]
                        rec = recp.tile([P, 4], F32, tag="rec")
                        nc.vector.reciprocal(rec[:], av[:, :, D])
                        for jj in range(4):
                            nc.vector.tensor_scalar(
                                osb[:, jj, x * D : (x + 1) * D],
                                av[:, jj, 0:D],
                                rec[:, jj : jj + 1], None, op0=MULT,
                            )

                    return tail

                def out_flush(pr, c, box):
                    # Combined 2-head DMA (512B runs).
                    def flush():
                        nc.sync.dma_start(
                            Out_r[c, :, :, 2 * pr * D : (2 * pr + 2) * D],
                            box["osb"],
                        )

                    return flush

                for c in range(NC_CHUNK - 1, -1, -1):
                    strips = list(range(4 * c + 4))
                    if c == NC_CHUNK - 1:
                        # start at J=12 so the first sc only needs qt qtr 3
                        strips = strips[12:] + strips[8:12] + strips[:8]
                    do_av = "noav" not in probes
                    if do_av:
                        av_a = ps_av.tile([P, 4, P], F32, tag="av",
                                          name=f"avA{pr}{c}")
                        av_b = ps_av.tile([P, 4, P], F32, tag="av",
                                          name=f"avB{pr}{c}")
                        if not AV_DEFER:
                            for av in (av_a, av_b):
                                # zero-opener: one K=1 matmul spanning all
                                # four quarters opens the bank's group;
                                # every av_mm overlaps its bytes, so the
                                # scheduler orders the opener first.
                                nc.tensor.matmul(
                                    av[:, :, 0 : D + 1],
                                    zt[0:1, 0:P],
                                    zt[0:1, 0 : 4 * (D + 1)],
                                    start=True,
                                    stop=False,
                                    skip_group_check=True,
                                )
                            # mirror-part MMs (rows Jp >= 4c+4): mirrors
                            # exist since chunks > c are done, so these
                            # pieces are ready at chunk start.
                            for c_src in range(c + 1, NC_CHUNK):
                                for x, av in ((0, av_a), (1, av_b)):
                                    for i in range(4):
                                        pending.append((1, make_mir_piece(
                                            pr, q1, c, x, i, av, c_src)))
                    sc = emit_sc(c, strips[0])
                    for k, J in enumerate(strips):
                        est = esp.tile([P, 1024], BF16, tag="es",
                                       name=f"es{pr}_{c}_{J}")
                        es[pr, c, J] = est
                        if "noexp" not in probes:
                            nc.scalar.activation(est[:], sc[:], EXP,
                                                 scale=0.125)
                        else:
                            nc.vector.memset(est[:, 0:1], 1.0)
                        if k + 1 < len(strips):
                            sc = emit_sc(c, strips[k + 1])
                        if do_av and not AV_DEFER:
                            # direct AV matmuls for this strip, right behind
                            # the next sc in priority order
                            last = k == len(strips) - 1
                            for x, av in ((0, av_a), (1, av_b)):
                                for i in range(4):
                                    av_mm(pr, q1, c, x, i, av, J,
                                          stop=last and i == 3)
                        nheavy = sum(w for w, _ in pending)
                        drain(3 if nheavy > 14 else 2 if nheavy > 6 else 1)
                        if c > J // 4:
                            # mirror whole chunk: 8 block-transposes in one
                            # XBAR DMA -> tiles [4c+i, J] for later chunks
                            m = mirp.tile([P, 8, P], BF16, tag="mir",
                                          name=f"m{pr}_{J}_{c}")
                            mir[pr, J, c] = m
                            nc.sync.dma_start(m[:], est[:], transpose=True)
                    if do_av:
                        if AV_DEFER:
                            for x, av in ((0, av_a), (1, av_b)):
                                for i in range(4):
                                    for jlo in (0, NT // 2):
                                        pending.append((1, make_burst_half(
                                            pr, c, x, i, av, q1,
                                            jlo, jlo + NT // 2)))
                        box = {}
                        # tails pop after the av writers (FIFO) so all
                        # writers are emitted before the readers
                        pending.append((0, out_half(pr, c, 0, av_a, box)))
                        pending.append((0, out_half(pr, c, 1, av_b, box)))
                        pending.append((0, out_flush(pr, c, box)))
                if pr + 2 < NPAIR:
                    pending.extend(
                        (0, pc) for pc in make_prep(pr + 2, states[pr + 2]))
            for _, pc in pending:
                pc()

        if loop_n > 1:
            with tc.For_i(0, loop_n, 1):
                body()
        else:
            body()


_CACHED = {}


def _build(loop_n: int = 1, probe: str = ""):
    key = (loop_n, probe)
    if key in _CACHED:
        return _CACHED[key]
    nc = bacc.Bacc("TRN2", target_bir_lowering=False, debug=False)
    Qt = nc.dram_tensor("Qt", [NPAIR * P, S], BF16, kind="ExternalInput")
    Q1 = nc.dram_tensor("Q1", [NPAIR * P, NT * 2 * (D + 1)], BF16,
                        kind="ExternalInput")
    Out = nc.dram_tensor("Out", [S, HPC * D], F32, kind="ExternalOutput")
    with tile.TileContext(nc) as tc:
        _emit(tc, Out.ap(), Qt.ap(), Q1.ap(), loop_n=loop_n, probe=probe)
    nc.compile()
    _CACHED[key] = nc
    return nc


def _bf16(a):
    import ml_dtypes

    return np.asarray(a, dtype=np.float32).astype(ml_dtypes.bfloat16)


def prep_core(Q: np.ndarray, core: int) -> dict:
    """Host-side marshalling of core `core`'s shard into QT/Q1 layouts."""
    b, hg = core // 2, core % 2
    qc = np.asarray(Q[b, :, hg * 512 : (hg + 1) * 512], dtype=np.float32)
    # QT: [pr*128 + x*64 + d, s]
    qt = np.ascontiguousarray(qc.T)  # [512, 2048]
    # Q1: [pr*128 + p, n*130 + x*65 + dd], dd==64 -> 1.0
    q1 = np.ones((NPAIR, P, NT, 2, D + 1), dtype=np.float32)
    qr = qc.reshape(NT, P, NPAIR, 2, D)  # [n, p, pr, x, d]
    q1[:, :, :, :, :D] = qr.transpose(2, 1, 0, 3, 4)
    return {
        "Qt": _bf16(qt),
        "Q1": _bf16(q1.reshape(NPAIR * P, NT * 2 * (D + 1))),
    }


def kernel(Q: np.ndarray, K: np.ndarray, V: np.ndarray,
           _probe: str = "") -> np.ndarray:
    nc = _build(1, _probe)
    in_maps = [prep_core(Q, core) for core in range(N_CORES)]
    res = run_bass_kernel_spmd(nc, in_maps, list(range(N_CORES))).results
    out = np.empty((B, S, DMODEL), np.float32)
    for core in range(N_CORES):
        b, hg = core // 2, core % 2
        out[b, :, hg * 512 : (hg + 1) * 512] = res[core]["Out"]
    return out
